# revision 2
# baseline (speedup 1.0000x reference)
"""Trainium2 Bass kernel for a 4-layer GRU decoder with attention.

Model (per time step, teacher forcing over T=20 steps):
    e   = emb[token]                          [B,H]
    q   = h3 @ attn_w.T                       [B,H]
    aw  = softmax(q . enc)                    [B,S]
    ctx = aw . enc                            [B,H]
    l0:   gru([e|ctx], h0);  l1..l3: gru(h_prev_layer, h_l)
    logits = h3 @ out_w.T -> log_softmax      [B,V]

Sharding: pure data parallel over batch, B=128 -> 16 rows per core on 8
NeuronCores.  All weights replicated (bf16).  No collectives.

Device mapping notes:
 - Matmuls keep the 16 batch rows on the output partition dim (M=16) and
   stream [K,512] weight panels as the moving operand; lhsT operands are
   transposed activations ([K,16] tiles) produced by PE transposes.
 - Attention scores/context are block-diagonal batched matmuls over the
   (b,s) pairs; off-diagonal garbage is killed by adding a -3e4 mask via
   an identity-matmul inject before the softmax exp, so the exp+accum
   pass yields the correct per-row denominators directly.
 - sigmoid(x) = 0.5*tanh(x/2)+0.5 so the recurrence needs only the
   "exp_and_others" ACT table set (tanh+exp).  The n-gate hidden half of
   w_hh is pre-scaled by 0.5 on the host to fold the sigmoid affine.
 - The embedding part of the layer-0 gates is precomputed for all steps
   in one batched matmul (teacher forcing makes all tokens known) and
   injected into the per-step PSUM accumulation via an identity matmul.
 - Output projection is batched over all (t,b) rows at the end;
   log_softmax skips the max-subtraction (logits are O(1), exp is safe
   in fp32): one exp+accum pass, an ln, and a broadcast subtract.
"""

import sys

import numpy as np

for _p in ("/opt/trn_rl_repo",):
    if _p not in sys.path:
        sys.path.insert(0, _p)

import ml_dtypes

import concourse.bacc as bacc
import concourse.bass as bass  # noqa: F401
import concourse.mybir as mybir
from concourse import tile
from concourse.bass_utils import run_bass_kernel_spmd

BF16 = mybir.dt.bfloat16
F32 = mybir.dt.float32
AF = mybir.ActivationFunctionType
ALU = mybir.AluOpType
NPBF16 = ml_dtypes.bfloat16

B, S, T, H, V, L = 128, 64, 20, 512, 20000, 4
NCORES = 8
BC = B // NCORES            # 16 batch rows per core
G = 3 * H                   # 1536 gate dims
KT = H // 128               # 4 K-tiles of the hidden dim
BS = BC * S                 # 1024 (b,s) pairs per core
START_ID = 1
NEG = -30000.0              # mask value (representable in bf16)

_PROGRAM_CACHE = {}


def _vchunks():
    out, c = [], 0
    while c < V:
        n = min(512, V - c)
        out.append((c, n))
        c += n
    return out


def _emit_bank(nc, dst, writers):
    """Emit one PSUM accumulation group: writers = [(lhsT, rhs), ...]."""
    last = len(writers) - 1
    for i, (lhsT, rhs) in enumerate(writers):
        nc.tensor.matmul(dst, lhsT, rhs, start=(i == 0), stop=(i == last))


def build_program():
    nc = bacc.Bacc(None)

    # ---- DRAM I/O ----
    et = nc.dram_tensor("et", [H, T * BC], BF16, kind="ExternalInput")
    encT = nc.dram_tensor("encT", [H, BS], BF16, kind="ExternalInput")
    encbs = nc.dram_tensor("encbs", [BS, H], BF16, kind="ExternalInput")
    hT0p = nc.dram_tensor("hT0p", [H, L * BC], BF16, kind="ExternalInput")
    h0d = nc.dram_tensor("h0d", [L * BC, H], F32, kind="ExternalInput")
    whhT = nc.dram_tensor("whhT", [L, H, G], BF16, kind="ExternalInput")
    wihT = nc.dram_tensor("wihT", [L - 1, H, G], BF16, kind="ExternalInput")
    wxeT = nc.dram_tensor("wxeT", [H, G], BF16, kind="ExternalInput")
    wxcT = nc.dram_tensor("wxcT", [H, G], BF16, kind="ExternalInput")
    awTd = nc.dram_tensor("awTd", [H, H], BF16, kind="ExternalInput")
    owT = nc.dram_tensor("owT", [H, V], BF16, kind="ExternalInput")
    maskb = nc.dram_tensor("maskb", [BC, BS], BF16, kind="ExternalInput")
    i16f = nc.dram_tensor("i16f", [BC, BC], F32, kind="ExternalInput")
    i16b = nc.dram_tensor("i16b", [BC, BC], BF16, kind="ExternalInput")

    lp = nc.dram_tensor("lp", [T * BC, V], F32, kind="ExternalOutput")
    fh = nc.dram_tensor("fh", [L * BC, H], F32, kind="ExternalOutput")
    awo = nc.dram_tensor("awo", [T, BC, BS], F32, kind="ExternalOutput")

    gxe_d = nc.dram_tensor("gxe_d", [T * BC, G], BF16)  # scratch

    with tile.TileContext(nc) as tc:
        with tc.tile_pool(name="state", bufs=1) as spool:
            # ---- persistent state ----
            hT = spool.tile([128, KT, 3 * BC], BF16, tag="hT")
            XT = spool.tile([128, KT, (T + 1) * BC], BF16, tag="XT")
            for k in range(KT):
                nc.sync.dma_start(out=hT[:, k], in_=hT0p[128 * k:128 * (k + 1), 0:3 * BC])
                nc.sync.dma_start(out=XT[:, k, 0:BC],
                                  in_=hT0p[128 * k:128 * (k + 1), 3 * BC:4 * BC])
            hs = []
            for l in range(L):
                h_l = spool.tile([BC, H], F32, tag=f"h{l}")
                nc.sync.dma_start(out=h_l[:], in_=h0d[BC * l:BC * (l + 1), :])
                hs.append(h_l)

            with tc.tile_pool(name="const", bufs=1) as cpool:
                # ---- resident constants ----
                enc_t = cpool.tile([128, KT, BS], BF16, tag="encT")
                for k in range(KT):
                    nc.sync.dma_start(out=enc_t[:, k], in_=encT[128 * k:128 * (k + 1), :])
                enc_b = cpool.tile([128, BS // 128, H], BF16, tag="encbs")
                for k in range(BS // 128):
                    nc.sync.dma_start(out=enc_b[:, k], in_=encbs[128 * k:128 * (k + 1), :])
                whh_t = cpool.tile([128, L, KT, G], BF16, tag="whhT")
                for l in range(L):
                    for k in range(KT):
                        nc.sync.dma_start(out=whh_t[:, l, k],
                                          in_=whhT[l, 128 * k:128 * (k + 1), :])
                wih_t = cpool.tile([128, L - 1, KT, G], BF16, tag="wihT")
                for l in range(L - 1):
                    for k in range(KT):
                        nc.sync.dma_start(out=wih_t[:, l, k],
                                          in_=wihT[l, 128 * k:128 * (k + 1), :])
                wxc_t = cpool.tile([128, KT, G], BF16, tag="wxcT")
                for k in range(KT):
                    nc.sync.dma_start(out=wxc_t[:, k], in_=wxcT[128 * k:128 * (k + 1), :])
                awt_t = cpool.tile([128, KT, H], BF16, tag="awT")
                for k in range(KT):
                    nc.sync.dma_start(out=awt_t[:, k], in_=awTd[128 * k:128 * (k + 1), :])
                mask_t = cpool.tile([BC, BS], BF16, tag="mask")
                nc.sync.dma_start(out=mask_t[:], in_=maskb[:])
                i16f_t = cpool.tile([BC, BC], F32, tag="i16f")
                nc.sync.dma_start(out=i16f_t[:], in_=i16f[:])
                i16b_t = cpool.tile([BC, BC], BF16, tag="i16b")
                nc.sync.dma_start(out=i16b_t[:], in_=i16b[:])

                # ---- precompute embedding part of layer-0 gates ----
                with tc.tile_pool(name="pre", bufs=1) as prepool, \
                     tc.tile_pool(name="psum_pre", bufs=2, space="PSUM") as ppre:
                    et_t = prepool.tile([128, KT, T * BC], BF16, tag="et")
                    for k in range(KT):
                        nc.sync.dma_start(out=et_t[:, k], in_=et[128 * k:128 * (k + 1), :])
                    wxe_t = prepool.tile([128, KT, G], BF16, tag="wxeT")
                    for k in range(KT):
                        nc.sync.dma_start(out=wxe_t[:, k], in_=wxeT[128 * k:128 * (k + 1), :])
                    for m, mm in enumerate((128, 128, 64)):
                        gxe_sb = prepool.tile([128, G], BF16, tag="gxe_sb")
                        for c in range(3):
                            g_ps = ppre.tile([128, 512], F32, tag="pre_ps")
                            _emit_bank(nc, g_ps[:mm], [
                                (et_t[:, k, 128 * m:128 * m + mm],
                                 wxe_t[:, k, 512 * c:512 * (c + 1)])
                                for k in range(KT)])
                            nc.scalar.copy(gxe_sb[:mm, 512 * c:512 * (c + 1)], g_ps[:mm])
                        nc.sync.dma_start(out=gxe_d[128 * m:128 * m + mm, :],
                                          in_=gxe_sb[:mm])

                # ---- recurrence ----
                with tc.tile_pool(name="work", bufs=2) as wpool, \
                     tc.tile_pool(name="psum_rec", bufs=1, space="PSUM") as ppool:
                    for t in range(T):
                        h3T = XT[:, :, BC * t:BC * (t + 1)]

                        # q = h3 @ attn_w.T
                        q_ps = ppool.tile([BC, H], F32, tag="vec_ps")
                        _emit_bank(nc, q_ps[:],
                                   [(h3T[:, k], awt_t[:, k]) for k in range(KT)])
                        q_sb = wpool.tile([BC, H], F32, tag="q_sb")
                        nc.scalar.copy(q_sb[:], q_ps[:])
                        qT_ps = ppool.tile([128, KT, BC], F32, tag="tp_ps")
                        for k in range(KT):
                            nc.tensor.transpose(qT_ps[:, k],
                                                q_sb[:, 128 * k:128 * (k + 1)], i16f_t[:])
                        qT = wpool.tile([128, KT, BC], BF16, tag="qT")
                        nc.scalar.copy(qT[:], qT_ps[:])

                        # scores (block-diagonal) + mask inject
                        sc_ps = ppool.tile([BC, BS], F32, tag="sc_ps")
                        for c in range(BS // 512):
                            sl = slice(512 * c, 512 * (c + 1))
                            _emit_bank(nc, sc_ps[:, sl],
                                       [(qT[:, k], enc_t[:, k, sl]) for k in range(KT)]
                                       + [(i16b_t[:], mask_t[:, sl])])

                        # softmax over the masked row
                        exp_sb = wpool.tile([BC, BS], F32, tag="exp_sb")
                        ssum = wpool.tile([BC, 1], F32, tag="ssum")
                        nc.scalar.activation(exp_sb[:], sc_ps[:], AF.Exp, accum_out=ssum[:])
                        rcp = wpool.tile([BC, 1], F32, tag="rcp")
                        nc.vector.reciprocal(rcp[:], ssum[:])
                        aw_sb = wpool.tile([BC, BS], F32, tag="aw_sb")
                        nc.vector.tensor_scalar(aw_sb[:], exp_sb[:], rcp[:], None, ALU.mult)
                        nc.sync.dma_start(out=awo[t], in_=aw_sb[:])

                        # context = aw . enc (block-diagonal via transposed aw)
                        awT_ps = ppool.tile([128, BS // 128, BC], F32, tag="tp_ps")
                        for j in range(BS // 128):
                            nc.tensor.transpose(awT_ps[:, j],
                                                aw_sb[:, 128 * j:128 * (j + 1)], i16f_t[:])
                        awT = wpool.tile([128, BS // 128, BC], BF16, tag="awT")
                        nc.scalar.copy(awT[:], awT_ps[:])
                        ctx_ps = ppool.tile([BC, H], F32, tag="vec_ps")
                        _emit_bank(nc, ctx_ps[:],
                                   [(awT[:, j], enc_b[:, j]) for j in range(BS // 128)])
                        ctx_sb = wpool.tile([BC, H], F32, tag="ctx_sb")
                        nc.scalar.copy(ctx_sb[:], ctx_ps[:])
                        cxT_ps = ppool.tile([128, KT, BC], F32, tag="tp_ps")
                        for k in range(KT):
                            nc.tensor.transpose(cxT_ps[:, k],
                                                ctx_sb[:, 128 * k:128 * (k + 1)], i16f_t[:])
                        cxT = wpool.tile([128, KT, BC], BF16, tag="cxT")
                        nc.scalar.copy(cxT[:], cxT_ps[:])

                        gxe_t = wpool.tile([BC, G], BF16, tag="gxe_t")
                        nc.sync.dma_start(out=gxe_t[:], in_=gxe_d[BC * t:BC * (t + 1), :])

                        # ---- GRU layers ----
                        for l in range(L):
                            g_r = ppool.tile([BC, 512], F32, tag="g_r")
                            g_z = ppool.tile([BC, 512], F32, tag="g_z")
                            g_nx = ppool.tile([BC, 512], F32, tag="g_nx")
                            g_nh = ppool.tile([BC, 512], F32, tag="g_nh")

                            hTl = (XT[:, :, BC * t:BC * (t + 1)] if l == 3
                                   else hT[:, :, BC * l:BC * (l + 1)])

                            def hh(gi):
                                return [(hTl[:, k], whh_t[:, l, k, 512 * gi:512 * (gi + 1)])
                                        for k in range(KT)]

                            def ih(gi):
                                if l == 0:
                                    return ([(cxT[:, k], wxc_t[:, k, 512 * gi:512 * (gi + 1)])
                                             for k in range(KT)]
                                            + [(i16b_t[:], gxe_t[:, 512 * gi:512 * (gi + 1)])])
                                return [(hT[:, k, BC * (l - 1):BC * l],
                                         wih_t[:, l - 1, k, 512 * gi:512 * (gi + 1)])
                                        for k in range(KT)]

                            _emit_bank(nc, g_r[:], hh(0) + ih(0))
                            _emit_bank(nc, g_z[:], hh(1) + ih(1))
                            _emit_bank(nc, g_nh[:], hh(2))
                            _emit_bank(nc, g_nx[:], ih(2))

                            # sigma(x) = 0.5*tanh(x/2)+0.5
                            trz = wpool.tile([BC, 1024], F32, tag="trz")
                            nc.scalar.activation(trz[:, 0:512], g_r[:], AF.Tanh, scale=0.5)
                            nc.scalar.activation(trz[:, 512:1024], g_z[:], AF.Tanh, scale=0.5)
                            t1 = wpool.tile([BC, 512], F32, tag="t1")
                            # r*hn = (tanh(gr/2)+1) * (0.5*hn); w_hh n-half pre-scaled
                            nc.vector.scalar_tensor_tensor(
                                t1[:], trz[:, 0:512], 1.0, g_nh[:], ALU.add, ALU.mult)
                            npre = wpool.tile([BC, 512], F32, tag="npre")
                            nc.vector.tensor_add(npre[:], t1[:], g_nx[:])
                            n_sb = wpool.tile([BC, 512], F32, tag="n_sb")
                            nc.scalar.activation(n_sb[:], npre[:], AF.Tanh)
                            c_sb = wpool.tile([BC, 512], F32, tag="c_sb")
                            nc.vector.tensor_scalar(c_sb[:], trz[:, 512:1024], 1.0, 0.5,
                                                    ALU.add, ALU.mult)
                            d_sb = wpool.tile([BC, 512], F32, tag="d_sb")
                            nc.vector.tensor_sub(d_sb[:], hs[l][:], n_sb[:])
                            u_sb = wpool.tile([BC, 512], F32, tag="u_sb")
                            nc.vector.tensor_mul(u_sb[:], c_sb[:], d_sb[:])
                            nc.vector.tensor_add(hs[l][:], n_sb[:], u_sb[:])

                            # transpose h'_l for the next matmuls
                            hT_ps = ppool.tile([128, KT, BC], F32, tag="tp_ps")
                            for k in range(KT):
                                nc.tensor.transpose(hT_ps[:, k],
                                                    hs[l][:, 128 * k:128 * (k + 1)],
                                                    i16f_t[:])
                            if l == 3:
                                nc.scalar.copy(XT[:, :, BC * (t + 1):BC * (t + 2)], hT_ps[:])
                            else:
                                nc.scalar.copy(hT[:, :, BC * l:BC * (l + 1)], hT_ps[:])

                    for l in range(L):
                        nc.sync.dma_start(out=fh[BC * l:BC * (l + 1), :], in_=hs[l][:])

            # ---- output projection + log_softmax ----
            with tc.tile_pool(name="oproj", bufs=1) as opool, \
                 tc.tile_pool(name="ow", bufs=6) as owpool, \
                 tc.tile_pool(name="ostg", bufs=4) as ostg, \
                 tc.tile_pool(name="psum_op", bufs=4, space="PSUM") as pop:
                chunks = _vchunks()
                for m, mm in enumerate((128, 128, 64)):
                    lg_sb = opool.tile([128, V], BF16, tag="lg_sb")
                    sums = opool.tile([128, len(chunks)], F32, tag="sums")
                    lhs_cols = slice(BC + 128 * m, BC + 128 * m + mm)
                    for ci, (c0, cn) in enumerate(chunks):
                        ow_k = owpool.tile([128, KT, 512], BF16, tag="ow")
                        for k in range(KT):
                            nc.sync.dma_start(out=ow_k[:, k, 0:cn],
                                              in_=owT[128 * k:128 * (k + 1), c0:c0 + cn])
                        lg_ps = pop.tile([128, 512], F32, tag="lg_ps")
                        _emit_bank(nc, lg_ps[:mm, 0:cn],
                                   [(XT[:, k, lhs_cols], ow_k[:, k, 0:cn])
                                    for k in range(KT)])
                        scr = ostg.tile([128, 512], F32, tag="scr")
                        nc.scalar.activation(scr[:mm, 0:cn], lg_ps[:mm, 0:cn], AF.Exp,
                                             accum_out=sums[:mm, ci:ci + 1])
                        nc.vector.tensor_copy(lg_sb[:mm, c0:c0 + cn], lg_ps[:mm, 0:cn])
                    ssum2 = opool.tile([128, 1], F32, tag="ssum2")
                    nc.vector.reduce_sum(ssum2[:mm], sums[:mm], axis=mybir.AxisListType.X)
                    logS = opool.tile([128, 1], F32, tag="logS")
                    nc.scalar.activation(logS[:mm], ssum2[:mm], AF.Ln)
                    for c0, cn in chunks:
                        stg = ostg.tile([128, 512], F32, tag="stg")
                        nc.vector.tensor_scalar(stg[:mm, 0:cn], lg_sb[:mm, c0:c0 + cn],
                                                logS[:mm], None, ALU.subtract)
                        nc.sync.dma_start(out=lp[128 * m:128 * m + mm, c0:c0 + cn],
                                          in_=stg[:mm, 0:cn])

    nc.compile()
    return nc


def _get_program():
    if "nc" not in _PROGRAM_CACHE:
        _PROGRAM_CACHE["nc"] = build_program()
    return _PROGRAM_CACHE["nc"]


def _bf(x):
    return np.ascontiguousarray(x.astype(np.float32)).astype(NPBF16)


def make_core_inputs(encoder_outputs, encoder_hidden, target_tensor, emb,
                     attn_w, w_ih0, w_ih_rest, w_hh, out_w):
    """Host-side prep: shard batch, transpose/cast weights."""
    tokens = np.concatenate(
        [np.full((B, 1), START_ID, dtype=np.int64),
         target_tensor[:, :-1].astype(np.int64)], axis=1)  # [B,T]

    whh = w_hh.copy()
    whh[:, 2 * H:3 * H, :] *= 0.5  # n-gate hidden half pre-scaled
    shared = {
        "whhT": _bf(whh.transpose(0, 2, 1)),
        "wihT": _bf(w_ih_rest.transpose(0, 2, 1)),
        "wxeT": _bf(w_ih0[:, :H].T),
        "wxcT": _bf(w_ih0[:, H:].T),
        "awTd": _bf(attn_w.T),
        "owT": _bf(out_w.T),
        "i16f": np.eye(BC, dtype=np.float32),
        "i16b": np.eye(BC).astype(NPBF16),
    }
    mask = np.full((BC, BS), NEG, np.float32)
    for b in range(BC):
        mask[b, S * b:S * (b + 1)] = 0.0
    shared["maskb"] = mask.astype(NPBF16)

    in_maps = []
    for c in range(NCORES):
        bsl = slice(BC * c, BC * (c + 1))
        toks = tokens[bsl]                     # [16, T]
        erows = emb[toks.T.reshape(-1)]        # [T*16, H], row = 16t+b
        enc_c = encoder_outputs[bsl]           # [16, S, H]
        eh_c = encoder_hidden[:, bsl]          # [L, 16, H]
        m = dict(shared)
        m["et"] = _bf(erows.T)
        m["encT"] = _bf(enc_c.reshape(BS, H).T)
        m["encbs"] = _bf(enc_c.reshape(BS, H))
        m["hT0p"] = _bf(eh_c.transpose(2, 0, 1).reshape(H, L * BC))
        m["h0d"] = np.ascontiguousarray(eh_c.reshape(L * BC, H), dtype=np.float32)
        in_maps.append(m)
    return in_maps


def kernel(encoder_outputs, encoder_hidden, target_tensor, emb, attn_w, attn_b,
           w_ih0, w_ih_rest, w_hh, b_ih, b_hh, out_w, out_b):
    encoder_outputs = np.asarray(encoder_outputs, np.float32)
    encoder_hidden = np.asarray(encoder_hidden, np.float32)
    target_tensor = np.asarray(target_tensor)
    emb = np.asarray(emb, np.float32)
    attn_w = np.asarray(attn_w, np.float32)
    w_ih0 = np.asarray(w_ih0, np.float32)
    w_ih_rest = np.asarray(w_ih_rest, np.float32)
    w_hh = np.asarray(w_hh, np.float32)
    out_w = np.asarray(out_w, np.float32)

    nc = _get_program()
    in_maps = make_core_inputs(encoder_outputs, encoder_hidden, target_tensor,
                               emb, attn_w, w_ih0, w_ih_rest, w_hh, out_w)
    res = run_bass_kernel_spmd(nc, in_maps, list(range(NCORES)))

    log_probs = np.empty((B, T, V), np.float32)
    final_h = np.empty((L, B, H), np.float32)
    attn = np.empty((B, T, S), np.float32)
    idx = np.arange(BC)
    for c in range(NCORES):
        bsl = slice(BC * c, BC * (c + 1))
        r = res.results[c]
        log_probs[bsl] = r["lp"].reshape(T, BC, V).transpose(1, 0, 2)
        final_h[:, bsl] = r["fh"].reshape(L, BC, H)
        aw = r["awo"].reshape(T, BC, BC, S)
        attn[bsl] = aw[:, idx, idx, :].transpose(1, 0, 2)
    return log_probs, final_h, attn
